# revision 21
# baseline (speedup 1.0000x reference)
"""VQ codebook encoding kernel for Trainium2, sharded over 8 NeuronCores.

Math (per shard of N tokens):
    l2[n,k]  = ||x_n - c_k||            (NOT squared)
    W        = softmax_k(l2 * scale_k)
    E[k,d]   = sum_n W[n,k] * (x[n,d] - c[k,d])
             = (W^T X)[k,d] - S_k * c[k,d],   S_k = sum_n W[n,k]

Key algebra: logits = s_k * l2 = sqrt(s_k^2 * l2^2), and
    s^2 l2^2 = (x . ct2s) + ccxx_s,
      ct2s[d,k]   = -2 c[k,d] s_k^2               (host, bf16)
      ccxx_s[n,k] = s_k^2 (||c_k||^2 + ||x_n||^2) (host, fp16)
so on-chip: A = ps + ccxx_s (one DVE add); EW = exp(exp(0.5 ln A));
W = EW * (1/den), with ln/exp/exp in ONE activation table.  The device
returns [M | S] = W^T [X|1] per chunk (separate PSUM banks + separate
output DMAs, so early chunks' outputs overlap later chunks' compute);
the host finishes E = sum M - (sum S) * codes.

Performance structure (the ACT engine's 9 transcendental passes are the
saturated resource; everything else hides under them):
- 3 ASYMMETRIC chunks of [6, 4, 6] tiles: chunk0 small enough that its
  score DMA lands early (ladder starts ~1us sooner than an 8-tile
  chunk), chunk2 sized so its softmax tail + aggregation + output DMA
  are all that remain at the end.
- each HWDGE queue (sync/scalar/gpsimd) carries at most 2 transfers:
  back-to-back transfers on one queue have a ~0.8us turnaround gap, so
  pieces are placed so each arrives just before its consumer.
- all matmul operands 16-bit (1 cycle/row on the PE vs 4 for fp32).
- fp16 for ccxx_s / A: values ~256 need ~0.03 abs accuracy for the
  logits; fp16 gives 0.05% rel.  A = s^2 l2^2 > 0 always (AM-GM,
  128-dim gaussians never collide), so ln never sees <= 0.
"""

import sys

if "/opt/trn_rl_repo" not in sys.path:
    sys.path.insert(0, "/opt/trn_rl_repo")

import numpy as np

N_CORES = 8
N, K, D = 16384, 32, 128
NPC = N // N_CORES          # tokens per core = 2048
NT = NPC // 128             # 128-token tiles per core = 16
XNW = 130                   # xn tile width: [x(128) | 1 | 0]

# chunk layout: (tile_offset, n_tiles).  Two chunks: the ACT engine's
# per-op fixed cost (~345ns) makes finer chunking net-slower.
CH = [(0, 8), (8, 8)]
NC_ = len(CH)

_CACHE = {}


def _force_combined_act_table(nc, mybir):
    """Seed hw_specs' cached activation-table dict so Ln/Exp resolve to
    the one set containing both ("natural_log_exp_and_others"), giving a
    single ACT_TABLE_LOAD."""
    import concourse.hw_specs as hw_specs

    AFT = mybir.ActivationFunctionType
    tables = hw_specs.get_activation_tables(nc.m.arch)
    if "natural_log_exp_and_others" not in tables:
        return
    for name, funcs in tables.items():
        if name != "natural_log_exp_and_others":
            funcs.discard(AFT.Exp)
            funcs.discard(AFT.Ln)
            funcs.discard(AFT.Square)


def _build_nc():
    import concourse.bacc as bacc
    import concourse.bass as bass
    import concourse.mybir as mybir

    f32 = mybir.dt.float32
    bf16 = mybir.dt.bfloat16
    f16 = mybir.dt.float16
    AFT = mybir.ActivationFunctionType
    ALU = mybir.AluOpType

    nc = bacc.Bacc(None, target_bir_lowering=False)
    _force_combined_act_table(nc, mybir)

    xa = nc.dram_tensor("xa", [128, K + NPC], bf16, kind="ExternalInput")
    cx = nc.dram_tensor("cx", [128, NT * K], f16, kind="ExternalInput")
    xn = nc.dram_tensor("xn", [128, NT * XNW], bf16, kind="ExternalInput")
    e_out = [
        nc.dram_tensor(f"E{c}", [K, XNW], f32, kind="ExternalOutput")
        for c in range(NC_)
    ]

    sb = lambda name, shape, dt: nc.alloc_sbuf_tensor(name, shape, dt)
    xa_sb = sb("xa_sb", [128, K + NPC], bf16)
    cx_sb = sb("cx_sb", [128, NT * K], f16)
    xn_sb = sb("xn_sb", [128, NT * XNW], bf16)
    a_t = [sb(f"a{c}", [128, n * K], f16) for c, (_, n) in enumerate(CH)]
    l_t = [sb(f"l{c}", [128, n * K], f32) for c, (_, n) in enumerate(CH)]
    t_t = [sb(f"t{c}", [128, n * K], f32) for c, (_, n) in enumerate(CH)]
    ew_t = [sb(f"ew{c}", [128, n * K], bf16) for c, (_, n) in enumerate(CH)]
    den = [sb(f"den{c}", [128, n], bf16) for c, (_, n) in enumerate(CH)]
    rden = [sb(f"rden{c}", [128, n], f32) for c, (_, n) in enumerate(CH)]
    w_t = [sb(f"w{c}", [128, n * K], bf16) for c, (_, n) in enumerate(CH)]
    e_sb = [sb(f"e_sb{c}", [K, XNW], f32) for c in range(NC_)]

    # full-bank PSUM allocations (scores + aggregation per chunk)
    ps = [nc.alloc_psum_tensor(f"ps{c}", [128, 512], f32) for c in range(NC_)]
    pms = [nc.alloc_psum_tensor(f"pms{c}", [K, 512], f32) for c in range(NC_)]

    ct2v = xa_sb[:, 0:K]

    def bck(apw, count):
        # [128, w] per-tile scalars -> [128, w, count] via stride-0 inner dim
        return bass.AP(
            tensor=apw.tensor,
            offset=apw.offset,
            ap=[list(apw.ap[0]), list(apw.ap[1]), [0, count]],
        )

    def t3(ap, k=K):
        return ap.rearrange("p (t k) -> p t k", k=k)

    qA = [nc.alloc_semaphore(f"qA{c}") for c in range(NC_)]  # xa per chunk
    qC = nc.alloc_semaphore("qC")        # ccxx_s
    qN = nc.alloc_semaphore("qN")        # xn
    mmS = nc.alloc_semaphore("mmS")      # PE: score matmuls done, per chunk
    dvA = nc.alloc_semaphore("dvA")      # DVE: A ready, per chunk
    ewN = nc.alloc_semaphore("ewN")      # ACT: EW ready, per chunk
    wR = nc.alloc_semaphore("wR")        # DVE: W ready, per chunk
    aggS = nc.alloc_semaphore("aggS")    # PE: aggregation done, per chunk
    eR = [nc.alloc_semaphore(f"eR{c}") for c in range(NC_)]
    oD = nc.alloc_semaphore("oD")

    def xa_slice(off, n):
        lo = 0 if off == 0 else K + off * 128
        hi = K + (off + n) * 128
        return xa[:, lo:hi], xa_sb[:, lo:hi]

    with nc.Block(no_gpsimd_drain=True) as block:

        @block.sync
        def _(sync):
            # chunk0 (with ct2s) alone on this queue; chunk1 rides scalar
            src, dst = xa_slice(*CH[0])
            sync.dma_start(out=dst, in_=src).then_inc(qA[0], 16)
            for c in range(NC_):
                sync.wait_ge(eR[c], 1)
                # No completion wait: block-exit drain fences the queue.
                sync.dma_start(out=e_out[c][:, :], in_=e_sb[c][:, :]).then_inc(oD, 16)

        @block.scalar
        def _(scalar):
            scalar.dma_start(out=cx_sb[:, :], in_=cx[:, :]).then_inc(qC, 16)
            src, dst = xa_slice(*CH[1])
            scalar.dma_start(out=dst, in_=src).then_inc(qA[1], 16)
            for c in range(NC_):
                scalar.wait_ge(dvA, c + 1)
                nc.scalar.activation(
                    out=l_t[c][:, :], in_=a_t[c][:, :], func=AFT.Ln
                )
                scalar.drain()
                nc.scalar.activation(
                    out=t_t[c][:, :], in_=l_t[c][:, :], func=AFT.Exp, scale=0.5
                )
                scalar.drain()
                nc.scalar.activation(
                    out=ew_t[c][:, :], in_=t_t[c][:, :], func=AFT.Exp
                ).then_inc(ewN)
            # chunk0 epilogue on the (now idle) scalar engine
            scalar.wait_ge(aggS, 1)
            nc.scalar.activation(
                out=e_sb[0][:, :], in_=pms[0][:, :XNW], func=AFT.Copy
            ).then_inc(eR[0])

        @block.gpsimd
        def _(gpsimd):
            gpsimd.dma_start(out=xn_sb[:, :], in_=xn[:, :]).then_inc(qN, 16)

        @block.tensor
        def _(tensor):
            for c, (off, n) in enumerate(CH):
                tensor.wait_ge(qA[c], 16)
                for i in range(n):
                    mm = nc.tensor.matmul(
                        ps[c][:, i * K : (i + 1) * K],
                        xa_sb[:, K + (off + i) * 128 : K + (off + i + 1) * 128],
                        ct2v, start=True, stop=True,
                    )
                    if i == n - 1:
                        mm.then_inc(mmS)
            tensor.wait_ge(qN, 16)
            # chunk0: one W batch; chunk1: W arrives in halves (wR 2 then 3)
            # so aggregation starts before the second half-multiply finishes.
            for c, (off, n) in enumerate(CH):
                h = n // 2
                for i in range(n):
                    if i == 0:
                        tensor.wait_ge(wR, 1 if c == 0 else 2)
                    if c == 1 and i == h:
                        tensor.wait_ge(wR, 3)
                    mm = nc.tensor.matmul(
                        pms[c][:, :XNW],
                        w_t[c][:, i * K : (i + 1) * K],
                        xn_sb[:, (off + i) * XNW : (off + i + 1) * XNW],
                        start=(i == 0), stop=(i == n - 1),
                    )
                    if i == n - 1:
                        mm.then_inc(aggS)

        @block.vector
        def _(vector):
            vector.wait_ge(qC, 16)
            for c, (off, n) in enumerate(CH):
                vector.wait_ge(mmS, c + 1)
                nc.vector.tensor_add(
                    a_t[c][:, :], ps[c][:, : n * K],
                    cx_sb[:, off * K : (off + n) * K],
                ).then_inc(dvA)
            for c, (off, n) in enumerate(CH):
                vector.wait_ge(ewN, c + 1)
                # bf16 den: per-token bias ~0.4% averages out across tokens
                # in E (~0.01% net) -- far inside the error budget.
                with nc.allow_low_precision(reason="bf16 softmax denominator"):
                    nc.vector.tensor_reduce(
                        out=den[c][:, :], in_=t3(ew_t[c][:, :]),
                        axis=mybir.AxisListType.X, op=ALU.add,
                    )
                vector.drain()
                nc.vector.reciprocal(out=rden[c][:, :], in_=den[c][:, :])
                vector.drain()
                if c == 0:
                    nc.vector.tensor_mul(
                        t3(w_t[c][:, :]), t3(ew_t[c][:, :]), bck(rden[c][:, :], K)
                    ).then_inc(wR)
                else:
                    # halves, so the aggregation can start on the first half
                    h = n // 2
                    nc.vector.tensor_mul(
                        t3(w_t[c][:, : h * K]),
                        t3(ew_t[c][:, : h * K]),
                        bck(rden[c][:, :h], K),
                    ).then_inc(wR)
                    nc.vector.tensor_mul(
                        t3(w_t[c][:, h * K :]),
                        t3(ew_t[c][:, h * K :]),
                        bck(rden[c][:, h:], K),
                    ).then_inc(wR)
            for c in range(1, NC_):
                vector.wait_ge(aggS, c + 1)
                nc.vector.tensor_copy(e_sb[c][:, :], pms[c][:, :XNW]).then_inc(eR[c])

    nc.compile()
    return nc


def _get_nc():
    if "nc" not in _CACHE:
        _CACHE["nc"] = _build_nc()
    return _CACHE["nc"]


def _prep_inputs(x, codes, scale):
    """Build the per-core input maps (all host-side numpy)."""
    import ml_dtypes

    bf16 = ml_dtypes.bfloat16

    x = np.asarray(x, dtype=np.float32).reshape(N, D)
    codes = np.asarray(codes, dtype=np.float32)
    scale = np.asarray(scale, dtype=np.float32)

    s2 = (scale * scale).astype(np.float32)                         # [K]
    ct2s = np.ascontiguousarray(-2.0 * codes.T * s2[None, :])       # [D, K]
    ccs = ((codes * codes).sum(axis=1) * s2).astype(np.float32)     # [K]

    in_maps = []
    for core in range(N_CORES):
        xs = x[core * NPC : (core + 1) * NPC]                       # [2048, 128]
        a = xs.reshape(128, NT, D)                                  # [p, t, d]
        xx = (a * a).sum(axis=2)                                    # [p, t]
        ccxx = xx[:, :, None] * s2[None, None, :] + ccs[None, None, :]
        xtp = np.ascontiguousarray(a.transpose(2, 1, 0)).reshape(128, NPC)
        xav = np.concatenate([ct2s, xtp], axis=1)                   # [128, K+NPC]
        xnv = np.concatenate(
            [
                a,
                np.ones((128, NT, 1), dtype=np.float32),
                np.zeros((128, NT, 1), dtype=np.float32),
            ],
            axis=2,
        ).reshape(128, NT * XNW)
        in_maps.append(
            {
                "xa": np.ascontiguousarray(xav.astype(bf16)),
                "cx": np.ascontiguousarray(
                    ccxx.reshape(128, NT * K).astype(np.float16)
                ),
                "xn": np.ascontiguousarray(xnv.astype(bf16)),
            }
        )
    return in_maps


def _finish(results, codes):
    """Host-side epilogue: E = sum M - (sum S) * codes."""
    codes = np.asarray(codes, dtype=np.float32)
    acc = np.zeros((K, XNW), dtype=np.float64)
    for r in results:
        for c in range(NC_):
            acc += np.asarray(r[f"E{c}"], dtype=np.float64)
    out = acc[:, :D] - acc[:, D : D + 1] * codes.astype(np.float64)
    return out.astype(np.float32)


def kernel(x, codes, scale):
    from concourse.bass_utils import run_bass_kernel_spmd

    nc = _get_nc()
    in_maps = _prep_inputs(x, codes, scale)
    res = run_bass_kernel_spmd(nc, in_maps, core_ids=list(range(N_CORES)))
    return _finish(res.results, codes)
